# revision 1
# baseline (speedup 1.0000x reference)
"""Trainium2 Bass kernel for an AttentionBlock (GroupNorm + QKV + MHA + proj + residual).

Shapes (hardcoded): x (4, 512, 2048) fp32, 8 heads, 32 groups, eps 1e-5.

Sharding over 8 cores: core c handles batch b = c//2 and 4 of the 8 heads
(h0 = 4*(c%2)). Each core computes groupnorm(x[b]) (replicated within the
batch pair -- cheap), the qkv rows for its own heads, attention for its 4
heads, and a *partial* projection (contraction over its 256 a-channels).
The two partials of each batch are summed on the host; the even core of the
pair also adds the residual x and the projection bias.

Device-side math notes:
  - norm_w / norm_b are folded into the qkv weights/bias on the host.
  - the attention scale (1/sqrt(sqrt(64))) is folded into Wq/Wk/bq/bk.
  - the v-bias contribution is folded into the proj bias (softmax rows sum
    to 1, so it's a constant per-channel shift of `a`).
  - scores are computed transposed: wT[s,t] = k^T q, so softmax's reduce
    axis s lands on the PSUM partition axis; the row-sums come for free as
    a 65th output row of the PV matmul (ones column appended to v^T), and
    1/rowsum is computed as exp(-ln(rowsum)) on the scalar engine (same
    activation table set as the softmax exp).
"""

import math
import os

import numpy as np

os.environ.setdefault("MYCRO_LOCAL_CACHE", "1")

B, C, T = 4, 512, 2048
HEADS = 8
GROUPS = 32
EPS = 1e-5
CH = C // HEADS           # 64 channels per head
HPC = 4                   # heads per core
NCORES = 8
GSIZE = C // GROUPS       # 16 channels per group (8 groups per 128-row tile)
INV_N = 1.0 / (GSIZE * T)
SCALE = 1.0 / math.sqrt(math.sqrt(CH))

_NC = None


def build_program():
    from contextlib import ExitStack

    import concourse.bass as bass  # noqa: F401
    import concourse.tile as tile
    from concourse import bacc, mybir

    f32 = mybir.dt.float32
    AF = mybir.ActivationFunctionType
    ALU = mybir.AluOpType
    AX = mybir.AxisListType

    nc = bacc.Bacc("TRN2", target_bir_lowering=False, debug=False,
                   num_devices=NCORES)

    def din(name, shape):
        return nc.dram_tensor(name, shape, f32, kind="ExternalInput").ap()

    x_gn = din("x_gn", (C, T))
    x_res = din("x_res", (C, T))
    wq = din("wq", (C, 256))
    wk = din("wk", (C, 256))
    wv = din("wv", (C, 256))
    bqk = din("bqk", (128, 4))        # cols: bq half0, bq half1, bk h0, bk h1
    wp = din("wp", (256, C))
    pb = din("pb", (128, 4))          # proj bias partial, col m = out rows 128m..
    g1 = din("g1", (128, 8))          # partition -> group indicator
    g2 = din("g2", (8, 128))          # group -> partition indicator
    out = nc.dram_tensor("out", (C, T), f32, kind="ExternalOutput").ap()

    KT = C // 128                     # 4 contraction tiles over channels

    with tile.TileContext(nc) as tc, ExitStack() as ctx:
        perm = ctx.enter_context(tc.tile_pool(name="perm", bufs=1))

        # --- long-lived tensors ---
        wq_sb = perm.tile([128, KT, 256], f32, tag="wq")
        wk_sb = perm.tile([128, KT, 256], f32, tag="wk")
        wv_sb = perm.tile([128, KT, 256], f32, tag="wv")
        nc.sync.dma_start(out=wq_sb, in_=wq.rearrange("(kk p) c -> p kk c", p=128))
        nc.sync.dma_start(out=wk_sb, in_=wk.rearrange("(kk p) c -> p kk c", p=128))
        nc.sync.dma_start(out=wv_sb, in_=wv.rearrange("(kk p) c -> p kk c", p=128))
        wp_sb = perm.tile([128, 2, C], f32, tag="wp")
        nc.sync.dma_start(out=wp_sb, in_=wp.rearrange("(kk p) c -> p kk c", p=128))
        bqk_sb = perm.tile([128, 4], f32, tag="bqk")
        nc.sync.dma_start(out=bqk_sb, in_=bqk[:, :])
        pb_sb = perm.tile([128, 4], f32, tag="pb")
        nc.sync.dma_start(out=pb_sb, in_=pb[:, :])
        g1_sb = perm.tile([128, 8], f32, tag="g1")
        nc.sync.dma_start(out=g1_sb, in_=g1[:, :])
        g2_sb = perm.tile([8, 128], f32, tag="g2")
        nc.sync.dma_start(out=g2_sb, in_=g2[:, :])
        ones1 = perm.tile([1, CH], f32, tag="ones1")
        nc.vector.memset(ones1, 1.0)
        eps8 = perm.tile([8, 1], f32, tag="eps8")
        nc.vector.memset(eps8, EPS)

        q_sb = [perm.tile([128, T], f32, tag=f"q{m}", name=f"q{m}") for m in range(2)]
        k_sb = [perm.tile([128, T], f32, tag=f"k{m}", name=f"k{m}") for m in range(2)]
        # v^T blocks: [s-part 128, s-block 16, head 4, 64 v-cols + ones col]
        vt_sb = perm.tile([128, T // 128, HPC, CH + 1], f32, tag="vt")
        nc.gpsimd.memset(vt_sb, 1.0)
        a_sb = [perm.tile([128, T], f32, tag=f"a{m}", name=f"a{m}") for m in range(2)]

        with tc.tile_pool(name="hp", bufs=1) as hp:
            h_sb = [hp.tile([128, T], f32, tag=f"h{i}", name=f"h{i}") for i in range(KT)]

            # ---------------- phase 1: groupnorm ----------------
            with (
                tc.tile_pool(name="ph1", bufs=1) as ph1,
                tc.tile_pool(name="scr1", bufs=2) as scr1,
                tc.tile_pool(name="ps1", bufs=1, space="PSUM") as ps1,
            ):
                xg = [ph1.tile([128, T], f32, tag=f"xg{i}", name=f"xg{i}") for i in range(KT)]
                for i in range(KT):
                    nc.sync.dma_start(out=xg[i], in_=x_gn[128 * i:128 * (i + 1), :])
                sums = ph1.tile([128, 8], f32, tag="sums")
                for i in range(KT):
                    nc.vector.tensor_reduce(
                        out=sums[:, i:i + 1], in_=xg[i], axis=AX.X, op=ALU.add)
                    sq = scr1.tile([128, T], f32, tag="sq")
                    nc.scalar.activation(
                        out=sq, in_=xg[i], func=AF.Square,
                        accum_out=sums[:, 4 + i:5 + i])
                pst = ps1.tile([8, 8], f32, tag="pst")
                nc.tensor.matmul(pst[:, :], g1_sb[:, :], sums[:, :],
                                 start=True, stop=True)
                mv = ph1.tile([8, 8], f32, tag="mv")
                nc.vector.tensor_scalar_mul(mv, in0=pst, scalar1=INV_N)
                musq = ph1.tile([8, 4], f32, tag="musq")
                nc.vector.tensor_mul(musq, in0=mv[:, 0:4], in1=mv[:, 0:4])
                rb = ph1.tile([8, 8], f32, tag="rb")
                nc.vector.tensor_sub(rb[:, 0:4], in0=mv[:, 4:8], in1=musq)
                nc.scalar.activation(out=rb[:, 0:4], in_=rb[:, 0:4],
                                     func=AF.Sqrt, bias=eps8, scale=1.0)
                nc.vector.reciprocal(out=rb[:, 0:4], in_=rb[:, 0:4])
                negmu = ph1.tile([8, 4], f32, tag="negmu")
                nc.vector.tensor_mul(negmu, in0=mv[:, 0:4], in1=rb[:, 0:4])
                nc.vector.tensor_scalar_mul(rb[:, 4:8], in0=negmu, scalar1=-1.0)
                psb = ps1.tile([128, 8], f32, tag="psb")
                nc.tensor.matmul(psb[:, :], g2_sb[:, :], rb[:, :],
                                 start=True, stop=True)
                sbc = ph1.tile([128, 8], f32, tag="sbc")
                nc.vector.tensor_copy(sbc, psb)
                for i in range(KT):
                    nc.vector.tensor_scalar(
                        out=h_sb[i], in0=xg[i],
                        scalar1=sbc[:, i:i + 1], scalar2=sbc[:, 4 + i:5 + i],
                        op0=ALU.mult, op1=ALU.add)

            # ---------------- phase 2: qkv ----------------
            with (
                tc.tile_pool(name="ps2", bufs=1, space="PSUM") as ps2,
                tc.tile_pool(name="ps2v", bufs=2, space="PSUM") as ps2v,
            ):
                for wsb, bcol0, dst in ((wq_sb, 0, q_sb), (wk_sb, 2, k_sb)):
                    for m in range(2):
                        pq = [ps2.tile([128, 512], f32, tag=f"pq{t}", name=f"pq{t}")
                              for t in range(4)]
                        for kk in range(KT):
                            lhsT = wsb[:, kk, 128 * m:128 * (m + 1)]
                            for t in range(4):
                                nc.tensor.matmul(
                                    pq[t][:, :], lhsT,
                                    h_sb[kk][:, 512 * t:512 * (t + 1)],
                                    start=(kk == 0), stop=(kk == KT - 1))
                        for t in range(4):
                            nc.vector.tensor_scalar_add(
                                out=dst[m][:, 512 * t:512 * (t + 1)],
                                in0=pq[t],
                                scalar1=bqk_sb[:, bcol0 + m:bcol0 + m + 1])
                for j in range(T // 128):
                    pv = ps2v.tile([128, HPC * CH], f32, tag="pv")
                    for kk in range(KT):
                        nc.tensor.matmul(
                            pv[:, :], h_sb[kk][:, 128 * j:128 * (j + 1)],
                            wv_sb[:, kk, :],
                            start=(kk == 0), stop=(kk == KT - 1))
                    nc.vector.tensor_copy(
                        out=vt_sb[:, j, :, 0:CH],
                        in_=pv.rearrange("p (hh c) -> p hh c", hh=HPC))

        # ---------------- phase 3: attention ----------------
        xrp = ctx.enter_context(tc.tile_pool(name="xrp", bufs=1))
        xr = [xrp.tile([128, T], f32, tag=f"xr{m}", name=f"xr{m}") for m in range(KT)]
        for m in range(KT):
            nc.sync.dma_start(out=xr[m], in_=x_res[128 * m:128 * (m + 1), :])
        with (
            tc.tile_pool(name="pssc", bufs=2, space="PSUM") as pssc,
            tc.tile_pool(name="psa", bufs=1, space="PSUM") as psa,
            tc.tile_pool(name="ep", bufs=3) as ep,
            tc.tile_pool(name="rp", bufs=2) as rp,
        ):
            for hi in range(HPC):
                m, off = hi // 2, 64 * (hi % 2)
                qh = q_sb[m][off:off + 64, :]
                kh = k_sb[m][off:off + 64, :]
                pa = psa.tile([65, T], f32, tag="pa")
                for j in range(T // 128):
                    lhs_k = kh[:, 128 * j:128 * (j + 1)]
                    lhs_v = vt_sb[:, j, hi, :]
                    for cnk in range(2):
                        base = 1024 * cnk
                        psc = pssc.tile([128, 1024], f32, tag="sc")
                        for t2 in range(2):
                            nc.tensor.matmul(
                                psc[:, 512 * t2:512 * (t2 + 1)], lhs_k,
                                qh[:, base + 512 * t2:base + 512 * (t2 + 1)],
                                start=True, stop=True)
                        e = ep.tile([128, 1024], f32, tag="e")
                        nc.scalar.activation(out=e, in_=psc, func=AF.Exp)
                        for t2 in range(2):
                            nc.tensor.matmul(
                                pa[0:65, base + 512 * t2:base + 512 * (t2 + 1)],
                                lhs_v, e[:, 512 * t2:512 * (t2 + 1)],
                                start=(j == 0), stop=(j == T // 128 - 1))
                # 1/rowsum via exp(-ln(.)), then broadcast via K=1 matmul
                rs = rp.tile([1, T], f32, tag="rs")
                nc.vector.tensor_copy(rs, pa[64:65, :])
                lnt = rp.tile([1, T], f32, tag="ln")
                nc.scalar.activation(out=lnt, in_=rs, func=AF.Ln)
                ri = rp.tile([1, T], f32, tag="ri")
                nc.scalar.activation(out=ri, in_=lnt, func=AF.Exp, scale=-1.0)
                for cnk in range(2):
                    base = 1024 * cnk
                    pr = pssc.tile([64, 1024], f32, tag="sc")
                    for t2 in range(2):
                        nc.tensor.matmul(
                            pr[:, 512 * t2:512 * (t2 + 1)], ones1[:, :],
                            ri[0:1, base + 512 * t2:base + 512 * (t2 + 1)],
                            start=True, stop=True)
                    rsb = rp.tile([64, 1024], f32, tag="rsb")
                    nc.vector.tensor_copy(rsb, pr)
                    nc.vector.tensor_mul(
                        out=a_sb[m][off:off + 64, base:base + 1024],
                        in0=pa[0:64, base:base + 1024], in1=rsb)

        # ---------------- phase 4: partial proj + residual ----------------
        with (
            tc.tile_pool(name="ps4", bufs=1, space="PSUM") as ps4,
            tc.tile_pool(name="op", bufs=2) as op_,
        ):
            for m in range(KT):
                pp = [ps4.tile([128, 512], f32, tag=f"pp{t}", name=f"pp{t}")
                      for t in range(4)]
                for kk in range(2):
                    lhsT = wp_sb[:, kk, 128 * m:128 * (m + 1)]
                    for t in range(4):
                        nc.tensor.matmul(
                            pp[t][:, :], lhsT,
                            a_sb[kk][:, 512 * t:512 * (t + 1)],
                            start=(kk == 0), stop=(kk == 1))
                ot = op_.tile([128, T], f32, tag="ot")
                for t in range(4):
                    nc.vector.scalar_tensor_tensor(
                        out=ot[:, 512 * t:512 * (t + 1)], in0=pp[t],
                        scalar=pb_sb[:, m:m + 1],
                        in1=xr[m][:, 512 * t:512 * (t + 1)],
                        op0=ALU.add, op1=ALU.add)
                nc.sync.dma_start(out=out[128 * m:128 * (m + 1), :], in_=ot)

    nc.compile()
    return nc


def _get_nc():
    global _NC
    if _NC is None:
        _NC = build_program()
    return _NC


def make_in_maps(x, norm_w, norm_b, qkv_w, qkv_b, proj_w, proj_b):
    f = lambda a: np.ascontiguousarray(np.asarray(a, dtype=np.float32))
    x, norm_w, norm_b = f(x), f(norm_w), f(norm_b)
    qkv_w, qkv_b, proj_w, proj_b = f(qkv_w), f(qkv_b), f(proj_w), f(proj_b)

    wf = qkv_w * norm_w[None, :]            # fold norm scale
    bf = qkv_b + qkv_w @ norm_b             # fold norm bias

    g1 = np.zeros((128, 8), np.float32)
    g1[np.arange(128), np.arange(128) // GSIZE] = 1.0
    g2 = np.ascontiguousarray(g1.T)

    in_maps = []
    for c in range(NCORES):
        b = c // 2
        h0 = HPC * (c % 2)
        rows_q = np.concatenate(
            [np.arange(192 * h, 192 * h + CH) for h in range(h0, h0 + HPC)])
        rows_k = rows_q + CH
        rows_v = rows_q + 2 * CH
        wq_c = wf[rows_q] * SCALE           # (256, C)
        wk_c = wf[rows_k] * SCALE
        wv_c = wf[rows_v]
        bq_c = bf[rows_q] * SCALE
        bk_c = bf[rows_k] * SCALE
        bv_c = bf[rows_v]
        ch0 = 256 * (c % 2)
        wp_c = proj_w[:, ch0:ch0 + 256]     # (C, 256)
        pb_c = wp_c @ bv_c
        if c % 2 == 0:
            pb_c = pb_c + proj_b
        # cols: [bq0, bq1, bk0, bk1]
        bqk_in = np.concatenate(
            [bq_c.reshape(2, 128).T, bk_c.reshape(2, 128).T], axis=1)
        in_maps.append({
            "x_gn": x[b],
            "x_res": x[b] if c % 2 == 0 else np.zeros((C, T), np.float32),
            "wq": np.ascontiguousarray(wq_c.T),
            "wk": np.ascontiguousarray(wk_c.T),
            "wv": np.ascontiguousarray(wv_c.T),
            "bqk": np.ascontiguousarray(bqk_in),
            "wp": np.ascontiguousarray(wp_c.T),
            "pb": np.ascontiguousarray(pb_c.reshape(4, 128).T),
            "g1": g1,
            "g2": g2,
        })
    return in_maps


def kernel(x, norm_w, norm_b, qkv_w, qkv_b, proj_w, proj_b, trace=False):
    from concourse.bass_utils import run_bass_kernel_spmd

    in_maps = make_in_maps(x, norm_w, norm_b, qkv_w, qkv_b, proj_w, proj_b)
    nc = _get_nc()
    res = run_bass_kernel_spmd(nc, in_maps, core_ids=list(range(NCORES)),
                               trace=trace)
    kernel.last_results = res
    parts = [res.results[c]["out"] for c in range(NCORES)]
    out = np.stack([parts[2 * b] + parts[2 * b + 1] for b in range(B)])
    return out.astype(np.float32)



# revision 2
# speedup vs baseline: 2.7998x; 2.7998x over previous
"""Trainium2 Bass kernel for an AttentionBlock (GroupNorm + QKV + MHA + proj + residual).

Shapes (hardcoded): x (4, 512, 2048) fp32, 8 heads, 32 groups, eps 1e-5.

The dominant cost in this environment is the host<->device wire (axon
tunnel, ~50-60 MB/s round trip), not device compute (~0.5 ms/core). So the
kernel is organized to minimize bytes on the wire:

  - data-parallel over batch: core c handles batch b=c (4 cores), so x is
    shipped exactly once (no replication) and outputs are disjoint (no
    host-side partial sums);
  - everything on the wire is fp16 (x, weights, output) -- halves bytes;
    tolerance is 2e-2 absmax-rel, fp16 end-to-end lands ~1e-3;
  - each core runs the full 8-head attention for its batch, residual is
    added on device so the output is final.

Device-side math notes (carried over from the fp32 version):
  - norm_w / norm_b folded into the qkv weights/bias on the host;
  - attention scale folded into Wq/Wk/bq/bk;
  - v-bias folded into the proj bias (softmax rows sum to 1);
  - scores computed transposed (softmax reduce axis on PSUM partitions);
    row-sums come free as a 65th output row of the PV matmul (ones column
    in v^T); 1/rowsum via exp(-ln(.)) on the scalar engine;
  - matmuls run on fp16 operands (1 cycle/row on the PE vs 4 for fp32).
"""

import math
import os

import numpy as np

os.environ.setdefault("MYCRO_LOCAL_CACHE", "1")

B, C, T = 4, 512, 2048
HEADS = 8
GROUPS = 32
EPS = 1e-5
CH = C // HEADS           # 64 channels per head
NCORES = 4                # one batch per core
GSIZE = C // GROUPS       # 16 channels per group (8 groups per 128-row tile)
INV_N = 1.0 / (GSIZE * T)
SCALE = 1.0 / math.sqrt(math.sqrt(CH))
KT = C // 128             # 4 contraction tiles over channels

_NC = None


def build_program():
    from contextlib import ExitStack

    import concourse.bass as bass  # noqa: F401
    import concourse.tile as tile
    from concourse import bacc, mybir

    f32 = mybir.dt.float32
    f16 = mybir.dt.float16
    AF = mybir.ActivationFunctionType
    ALU = mybir.AluOpType
    AX = mybir.AxisListType

    nc = bacc.Bacc("TRN2", target_bir_lowering=False, debug=False,
                   num_devices=NCORES)

    x_in = nc.dram_tensor("x", (C, T), f16, kind="ExternalInput").ap()
    wqkv = nc.dram_tensor("wqkv", (C, 1024), f16, kind="ExternalInput").ap()
    wv = nc.dram_tensor("wv", (C, C), f16, kind="ExternalInput").ap()
    bqk = nc.dram_tensor("bqk", (128, 8), f32, kind="ExternalInput").ap()
    wp = nc.dram_tensor("wp", (C, C), f16, kind="ExternalInput").ap()
    pb = nc.dram_tensor("pb", (128, 4), f32, kind="ExternalInput").ap()
    g1 = nc.dram_tensor("g1", (128, 8), f32, kind="ExternalInput").ap()
    g2 = nc.dram_tensor("g2", (8, 128), f32, kind="ExternalInput").ap()
    out = nc.dram_tensor("out", (C, T), f16, kind="ExternalOutput").ap()

    with tile.TileContext(nc) as tc, ExitStack() as ctx:
        perm = ctx.enter_context(tc.tile_pool(name="perm", bufs=1))

        # --- long-lived tensors ---
        wqk_sb = perm.tile([128, KT, 1024], f16, tag="wqk")
        nc.sync.dma_start(out=wqk_sb, in_=wqkv.rearrange("(kk p) c -> p kk c", p=128))
        wv_sb = perm.tile([128, KT, C], f16, tag="wv")
        nc.sync.dma_start(out=wv_sb, in_=wv.rearrange("(kk p) c -> p kk c", p=128))
        wp_sb = perm.tile([128, KT, C], f16, tag="wp")
        nc.sync.dma_start(out=wp_sb, in_=wp.rearrange("(kk p) c -> p kk c", p=128))
        bqk_sb = perm.tile([128, 8], f32, tag="bqk")
        nc.sync.dma_start(out=bqk_sb, in_=bqk[:, :])
        pb_sb = perm.tile([128, 4], f32, tag="pb")
        nc.sync.dma_start(out=pb_sb, in_=pb[:, :])
        g1_sb = perm.tile([128, 8], f32, tag="g1")
        nc.sync.dma_start(out=g1_sb, in_=g1[:, :])
        g2_sb = perm.tile([8, 128], f32, tag="g2")
        nc.sync.dma_start(out=g2_sb, in_=g2[:, :])
        ones1 = perm.tile([1, CH], f16, tag="ones1")
        nc.vector.memset(ones1, 1.0)
        eps8 = perm.tile([8, 1], f32, tag="eps8")
        nc.vector.memset(eps8, EPS)

        xg = [perm.tile([128, T], f16, tag=f"xg{i}", name=f"xg{i}")
              for i in range(KT)]
        for i in range(KT):
            nc.sync.dma_start(out=xg[i], in_=x_in[128 * i:128 * (i + 1), :])

        q_sb = [perm.tile([128, T], f16, tag=f"q{m}", name=f"q{m}") for m in range(KT)]
        k_sb = [perm.tile([128, T], f16, tag=f"k{m}", name=f"k{m}") for m in range(KT)]
        # v^T blocks: [s-part 128, s-block 16, head 8, 64 v-cols + ones col]
        vt_sb = perm.tile([128, T // 128, HEADS, CH + 1], f16, tag="vt")
        nc.gpsimd.memset(vt_sb, 1.0)
        a_sb = [perm.tile([128, T], f16, tag=f"a{m}", name=f"a{m}") for m in range(KT)]

        with tc.tile_pool(name="hp", bufs=1) as hp:
            h_sb = [hp.tile([128, T], f16, tag=f"h{i}", name=f"h{i}") for i in range(KT)]

            # ---------------- phase 1: groupnorm ----------------
            with (
                tc.tile_pool(name="ph1", bufs=1) as ph1,
                tc.tile_pool(name="scr1", bufs=2) as scr1,
                tc.tile_pool(name="ps1", bufs=1, space="PSUM") as ps1,
            ):
                sums = ph1.tile([128, 8], f32, tag="sums")
                for i in range(KT):
                    nc.vector.tensor_reduce(
                        out=sums[:, i:i + 1], in_=xg[i], axis=AX.X, op=ALU.add)
                    sq = scr1.tile([128, T], f16, tag="sq")
                    nc.scalar.activation(
                        out=sq, in_=xg[i], func=AF.Square,
                        accum_out=sums[:, 4 + i:5 + i])
                pst = ps1.tile([8, 8], f32, tag="pst")
                nc.tensor.matmul(pst[:, :], g1_sb[:, :], sums[:, :],
                                 start=True, stop=True)
                mv = ph1.tile([8, 8], f32, tag="mv")
                nc.vector.tensor_scalar_mul(mv, in0=pst, scalar1=INV_N)
                musq = ph1.tile([8, 4], f32, tag="musq")
                nc.vector.tensor_mul(musq, in0=mv[:, 0:4], in1=mv[:, 0:4])
                rb = ph1.tile([8, 8], f32, tag="rb")
                nc.vector.tensor_sub(rb[:, 0:4], in0=mv[:, 4:8], in1=musq)
                nc.scalar.activation(out=rb[:, 0:4], in_=rb[:, 0:4],
                                     func=AF.Sqrt, bias=eps8, scale=1.0)
                nc.vector.reciprocal(out=rb[:, 0:4], in_=rb[:, 0:4])
                negmu = ph1.tile([8, 4], f32, tag="negmu")
                nc.vector.tensor_mul(negmu, in0=mv[:, 0:4], in1=rb[:, 0:4])
                nc.vector.tensor_scalar_mul(rb[:, 4:8], in0=negmu, scalar1=-1.0)
                psb = ps1.tile([128, 8], f32, tag="psb")
                nc.tensor.matmul(psb[:, :], g2_sb[:, :], rb[:, :],
                                 start=True, stop=True)
                sbc = ph1.tile([128, 8], f32, tag="sbc")
                nc.vector.tensor_copy(sbc, psb)
                for i in range(KT):
                    nc.vector.tensor_scalar(
                        out=h_sb[i], in0=xg[i],
                        scalar1=sbc[:, i:i + 1], scalar2=sbc[:, 4 + i:5 + i],
                        op0=ALU.mult, op1=ALU.add)

            # ---------------- phase 2: qkv ----------------
            with (
                tc.tile_pool(name="ps2", bufs=1, space="PSUM") as ps2,
                tc.tile_pool(name="ps2v", bufs=2, space="PSUM") as ps2v,
            ):
                for m in range(8):            # 4 q tiles then 4 k tiles
                    dst = q_sb[m] if m < 4 else k_sb[m - 4]
                    pq = [ps2.tile([128, 512], f32, tag=f"pq{t}", name=f"pq{t}")
                          for t in range(4)]
                    for kk in range(KT):
                        lhsT = wqk_sb[:, kk, 128 * m:128 * (m + 1)]
                        for t in range(4):
                            nc.tensor.matmul(
                                pq[t][:, :], lhsT,
                                h_sb[kk][:, 512 * t:512 * (t + 1)],
                                start=(kk == 0), stop=(kk == KT - 1))
                    for t in range(4):
                        nc.vector.tensor_scalar_add(
                            out=dst[:, 512 * t:512 * (t + 1)],
                            in0=pq[t],
                            scalar1=bqk_sb[:, m:m + 1])
                for j in range(T // 128):
                    pv = ps2v.tile([128, C], f32, tag="pv")
                    for kk in range(KT):
                        nc.tensor.matmul(
                            pv[:, :], h_sb[kk][:, 128 * j:128 * (j + 1)],
                            wv_sb[:, kk, :],
                            start=(kk == 0), stop=(kk == KT - 1))
                    nc.vector.tensor_copy(
                        out=vt_sb[:, j, :, 0:CH],
                        in_=pv.rearrange("p (hh c) -> p hh c", hh=HEADS))

        # ---------------- phase 3: attention ----------------
        with (
            tc.tile_pool(name="pssc", bufs=2, space="PSUM") as pssc,
            tc.tile_pool(name="psa", bufs=1, space="PSUM") as psa,
            tc.tile_pool(name="ep", bufs=3) as ep,
            tc.tile_pool(name="rp", bufs=2) as rp,
        ):
            for hi in range(HEADS):
                m, off = hi // 2, CH * (hi % 2)
                qh = q_sb[m][off:off + CH, :]
                kh = k_sb[m][off:off + CH, :]
                pa = psa.tile([CH + 1, T], f32, tag="pa")
                for j in range(T // 128):
                    lhs_k = kh[:, 128 * j:128 * (j + 1)]
                    lhs_v = vt_sb[:, j, hi, :]
                    for cnk in range(2):
                        base = 1024 * cnk
                        psc = pssc.tile([128, 1024], f32, tag="sc")
                        for t2 in range(2):
                            nc.tensor.matmul(
                                psc[:, 512 * t2:512 * (t2 + 1)], lhs_k,
                                qh[:, base + 512 * t2:base + 512 * (t2 + 1)],
                                start=True, stop=True)
                        e = ep.tile([128, 1024], f16, tag="e")
                        nc.scalar.activation(out=e, in_=psc, func=AF.Exp)
                        for t2 in range(2):
                            nc.tensor.matmul(
                                pa[0:CH + 1, base + 512 * t2:base + 512 * (t2 + 1)],
                                lhs_v, e[:, 512 * t2:512 * (t2 + 1)],
                                start=(j == 0), stop=(j == T // 128 - 1))
                # 1/rowsum via exp(-ln(.)), then broadcast via K=1 matmul
                rs = rp.tile([1, T], f32, tag="rs")
                nc.vector.tensor_copy(rs, pa[CH:CH + 1, :])
                lnt = rp.tile([1, T], f32, tag="ln")
                nc.scalar.activation(out=lnt, in_=rs, func=AF.Ln)
                ri = rp.tile([1, T], f16, tag="ri")
                nc.scalar.activation(out=ri, in_=lnt, func=AF.Exp, scale=-1.0)
                for cnk in range(2):
                    base = 1024 * cnk
                    pr = pssc.tile([CH, 1024], f32, tag="sc")
                    for t2 in range(2):
                        nc.tensor.matmul(
                            pr[:, 512 * t2:512 * (t2 + 1)], ones1[:, :],
                            ri[0:1, base + 512 * t2:base + 512 * (t2 + 1)],
                            start=True, stop=True)
                    rsb = rp.tile([CH, 1024], f16, tag="rsb")
                    nc.vector.tensor_copy(rsb, pr)
                    nc.vector.tensor_mul(
                        out=a_sb[m][off:off + CH, base:base + 1024],
                        in0=pa[0:CH, base:base + 1024], in1=rsb)

        # ---------------- phase 4: proj + residual ----------------
        with (
            tc.tile_pool(name="ps4", bufs=1, space="PSUM") as ps4,
            tc.tile_pool(name="op", bufs=2) as op_,
        ):
            for m in range(KT):
                pp = [ps4.tile([128, 512], f32, tag=f"pp{t}", name=f"pp{t}")
                      for t in range(4)]
                for kk in range(KT):
                    lhsT = wp_sb[:, kk, 128 * m:128 * (m + 1)]
                    for t in range(4):
                        nc.tensor.matmul(
                            pp[t][:, :], lhsT,
                            a_sb[kk][:, 512 * t:512 * (t + 1)],
                            start=(kk == 0), stop=(kk == KT - 1))
                ot = op_.tile([128, T], f16, tag="ot")
                for t in range(4):
                    nc.vector.scalar_tensor_tensor(
                        out=ot[:, 512 * t:512 * (t + 1)], in0=pp[t],
                        scalar=pb_sb[:, m:m + 1],
                        in1=xg[m][:, 512 * t:512 * (t + 1)],
                        op0=ALU.add, op1=ALU.add)
                nc.sync.dma_start(out=out[128 * m:128 * (m + 1), :], in_=ot)

    nc.compile()
    return nc


def _get_nc():
    global _NC
    if _NC is None:
        _NC = build_program()
    return _NC


def make_in_maps(x, norm_w, norm_b, qkv_w, qkv_b, proj_w, proj_b):
    f = lambda a: np.asarray(a, dtype=np.float32)
    x = f(x)
    norm_w, norm_b = f(norm_w), f(norm_b)
    qkv_w, qkv_b, proj_w, proj_b = f(qkv_w), f(qkv_b), f(proj_w), f(proj_b)

    wf = qkv_w * norm_w[None, :]            # fold norm scale
    bf = qkv_b + qkv_w @ norm_b             # fold norm bias

    # torch reshape-before-split row order: head h -> rows 192h+[0:64) q,
    # [64:128) k, [128:192) v
    rows_q = (np.arange(HEADS)[:, None] * 3 * CH + np.arange(CH)[None, :]).ravel()
    rows_k = rows_q + CH
    rows_v = rows_q + 2 * CH

    wqk = np.concatenate([wf[rows_q], wf[rows_k]], axis=0) * SCALE  # (1024, C)
    wqkv_in = np.ascontiguousarray(wqk.T, dtype=np.float16)         # (C, 1024)
    wv_in = np.ascontiguousarray(wf[rows_v].T, dtype=np.float16)    # (C, C)
    bqk_in = np.ascontiguousarray(
        (np.concatenate([bf[rows_q], bf[rows_k]]) * SCALE)
        .reshape(8, 128).T.astype(np.float32))                      # (128, 8)
    wp_in = np.ascontiguousarray(proj_w.T, dtype=np.float16)        # (C, C)
    pb_in = np.ascontiguousarray(
        (proj_b + proj_w @ bf[rows_v]).reshape(4, 128).T.astype(np.float32))

    g1 = np.zeros((128, 8), np.float32)
    g1[np.arange(128), np.arange(128) // GSIZE] = 1.0
    g2 = np.ascontiguousarray(g1.T)

    x16 = x.astype(np.float16)

    in_maps = []
    for c in range(NCORES):
        in_maps.append({
            "x": x16[c],
            "wqkv": wqkv_in,
            "wv": wv_in,
            "bqk": bqk_in,
            "wp": wp_in,
            "pb": pb_in,
            "g1": g1,
            "g2": g2,
        })
    return in_maps


def kernel(x, norm_w, norm_b, qkv_w, qkv_b, proj_w, proj_b, trace=False):
    from concourse.bass_utils import run_bass_kernel_spmd

    in_maps = make_in_maps(x, norm_w, norm_b, qkv_w, qkv_b, proj_w, proj_b)
    nc = _get_nc()
    res = run_bass_kernel_spmd(nc, in_maps, core_ids=list(range(NCORES)),
                               trace=trace)
    kernel.last_results = res
    out = np.stack([res.results[c]["out"] for c in range(NCORES)])
    return out.astype(np.float32)
